# revision 68
# baseline (speedup 1.0000x reference)
"""Trainium2 Bass kernel for the blockwise spiking network (nn_Blocks_86096914416140).

Sharding: data-parallel over batch B=32 across 8 NeuronCores (4 batches/core),
all parameters replicated, zero collectives.

v1 design: the 4 batches per core form 2 independent pairs (the recurrence
couples channels, never batches), software-pipelined one block apart so the
serial spike->matmul->membrane chain of one pair overlaps the other's work:

  PE    : xr_u = x + W@spk_u(n-1) + beta*v_init@t0  (x-identity matmul
          pre-issued one block ahead so the burst is just W + vinit)
  DVE   : cur = q*xr ; mem = scan(beta,cur) ; fs = (mem-1)>thr into a
          33-wide segment layout with zero spacer columns ;
          s = or-scan over the 33-layout [bf16] ;
          spk = s xor shift(s) in ONE 2x-mode bf16 op (the shifted view
          reads each segment's spacer zero as its t=-1 value) ;
          q = nsr|s [bf16 2x] ; pdec = scan(p, spk)
  Pool  : ns/v_init/adaptation smalls ; thr' = a'*b*p^{t+1}
          (GpSimd has no ScalarTensorTensor/scan opcodes on TRN2 - only
          tensor_tensor / tensor_single_scalar / copy forms are legal)
  Act   : nsr = 1-s_last ; spike regroup to f32 out groups ; out stores
          ride the Act HWDGE queue, x loads the SP queue

All float arithmetic is bit-identical to the flips=0 baseline (the dynamics
are chaotic: mem perturbations ~1e-5 cascade to >5000 output flips, so the
fp32 scan chain, the exact 3-way bf16 weight split, and the PSUM accumulation
order are all preserved).  The replaced pieces (count-scan -> or-scan,
spk = first-spike edge via xor, gate = s|ns) are exact boolean identities.
Measured: 167096 ns cost-model exec (baseline 196394), flips=0 on HW.
"""

import numpy as np

B, C, T_LEN, T = 32, 512, 1024, 32
NB = T_LEN // T          # 32 blocks
NCORES = 8
BPC = B // NCORES        # 4 batches per core
CH = C // 128            # 4 channel tiles
NP = 2                   # batch pairs per core
PB = BPC // NP           # batches per pair (2)
PFREE = CH * PB * T      # 256 free elements per pair tile
NSEG = CH * PB           # 8 (c,b) segments per pair tile
SEGF = NSEG * (T + 1)    # 264: segment layout with a zero spacer column
GRP = 8                  # blocks per x-load group

_compiled = None


def _build_program():
    import concourse.bass as bass
    import concourse.bacc as bacc
    import concourse.tile as tile
    from concourse import mybir
    from concourse._compat import with_exitstack
    from contextlib import ExitStack

    f32 = mybir.dt.float32
    bf16 = mybir.dt.bfloat16
    Alu = mybir.AluOpType

    nc = bacc.Bacc()
    x_d = nc.declare_dram_parameter("x_sh", [BPC, C, T_LEN], f32, isOutput=False)
    wt_d = nc.declare_dram_parameter("wt", [128, 48, 128], bf16, isOutput=False)
    betat_d = nc.declare_dram_parameter("betat", [128, CH], f32, isOutput=False)
    ident_d = nc.declare_dram_parameter("ident", [128, 128], f32, isOutput=False)
    betaseg_d = nc.declare_dram_parameter("betaseg", [128, PFREE], f32, isOutput=False)
    pseg_d = nc.declare_dram_parameter("pseg", [128, PFREE], f32, isOutput=False)
    seg33_d = nc.declare_dram_parameter("seg33", [128, SEGF], f32, isOutput=False)
    bp1_d = nc.declare_dram_parameter("bp1", [128, PFREE], f32, isOutput=False)
    p32_d = nc.declare_dram_parameter("p32t", [128, CH], f32, isOutput=False)
    invp_d = nc.declare_dram_parameter("invpt", [128, CH], f32, isOutput=False)
    out_d = nc.declare_dram_parameter("out", [BPC, C, T_LEN], f32, isOutput=True)

    def dram_block_ap(handle, ci, nblk, nt):
        # [128 part = c_lo, (b, t)] view of dram[b, ci*128:(ci+1)*128, nblk*T:...]
        return bass.AP(
            tensor=handle,
            offset=ci * 128 * T_LEN + nblk * T,
            ap=[[T_LEN, 128], [C * T_LEN, BPC], [1, nt]],
        )

    def pf(t4):
        # [128, CH, PB, T] pair tile -> [128, 256] dense view
        return t4.rearrange("p c u t -> p (c u t)")

    @with_exitstack
    def kern(ctx: ExitStack, tc: tile.TileContext):
        consts = ctx.enter_context(tc.tile_pool(name="consts", bufs=1))
        xpool = ctx.enter_context(tc.tile_pool(name="xpool", bufs=3))
        work = ctx.enter_context(tc.tile_pool(name="work", bufs=3))
        spkp = ctx.enter_context(tc.tile_pool(name="spkp", bufs=3))
        small = ctx.enter_context(tc.tile_pool(name="small", bufs=3))
        psum = ctx.enter_context(tc.tile_pool(name="psum", bufs=3, space="PSUM"))

        dma = nc.sync

        x_groups = {}
        sgrps = {}

        def load_group(g):
            xg = xpool.tile([128, CH, BPC, GRP * T], f32, tag="xg")
            for ci in range(CH):
                dma.dma_start(out=xg[:, ci],
                              in_=dram_block_ap(x_d, ci, g * GRP, GRP * T))
            x_groups[g] = xg

        load_group(0)

        # SP-queue order tracks first-use time: block0 chain tables, then
        # the v1 weight chunk (first matmuls), remaining tables, then v2/v3.
        # wt is partition-major in DRAM (contiguous 4KB runs, no small-
        # element DMA penalty) and split in 3 so the first burst can start
        # before the whole 12KB/partition transfer lands.
        betaseg_t = consts.tile([128, PFREE], f32, tag="betaseg")
        dma.dma_start(out=betaseg_t[:], in_=betaseg_d[:])
        seg33_t = consts.tile([128, SEGF], f32, tag="seg33")
        dma.dma_start(out=seg33_t[:], in_=seg33_d[:])
        wt_t = consts.tile([128, 48, 128], bf16, tag="wt")
        dma.dma_start(out=wt_t[:, 0:16], in_=wt_d[:, 0:16])
        id_t = consts.tile([128, 128], f32, tag="ident")
        nc.scalar.dma_start(out=id_t[:], in_=ident_d[:])
        betat_t = consts.tile([128, CH, 1, 1], f32, tag="betat")
        dma.dma_start(out=betat_t[:],
                      in_=betat_d[:].rearrange("p (c u v) -> p c u v", u=1, v=1))
        pseg_t = consts.tile([128, PFREE], f32, tag="pseg")
        dma.dma_start(out=pseg_t[:], in_=pseg_d[:])
        bp1_t = consts.tile([128, CH, PB, T], f32, tag="bp1")
        dma.dma_start(out=bp1_t[:],
                      in_=bp1_d[:].rearrange("p (c u t) -> p c u t", c=CH, u=PB))
        p32_t = consts.tile([128, CH, 1, 1], f32, tag="p32t")
        dma.dma_start(out=p32_t[:],
                      in_=p32_d[:].rearrange("p (c u v) -> p c u v", u=1, v=1))
        invp_t = consts.tile([128, CH, 1, 1], f32, tag="invpt")
        dma.dma_start(out=invp_t[:],
                      in_=invp_d[:].rearrange("p (c u v) -> p c u v", u=1, v=1))
        dma.dma_start(out=wt_t[:, 16:32], in_=wt_d[:, 16:32])
        dma.dma_start(out=wt_t[:, 32:48], in_=wt_d[:, 32:48])

        thr0_t = consts.tile([128, CH, PB, T], f32, tag="thr0")
        nc.vector.memset(thr0_t[:], 0.0)
        fs33 = []
        s33 = []
        for u in range(NP):
            f33u = consts.tile([128, NSEG, T + 1], f32, tag=f"fs33_{u}")
            nc.vector.memset(f33u[:], 0.0)
            fs33.append(f33u)
            s33u = consts.tile([128, 1 + SEGF], bf16, tag=f"s33_{u}")
            nc.vector.memset(s33u[:], 0.0)
            s33.append(s33u)

        # persistent per-pair state (rebound each block)
        spk_prev = [None, None]
        q_t = [None, None]
        thr_t = [thr0_t, thr0_t]
        vb_t = [None, None]
        a_t = [None, None]
        xr_next = [None, None]   # next block's PSUM tile, x pre-injected

        def issue_block(u, n):
            gi, go = divmod(n, GRP)
            if u == 0:
                # pair 0 leads: it owns group prefetch and sgrp allocation
                if go == 0 and gi not in sgrps:
                    sgrp_new = xpool.tile([128, CH, BPC, GRP, T], f32,
                                          tag="sgrp")
                    sgrps[gi] = sgrp_new
                if go == 2 and (gi + 1) * GRP < NB and gi + 1 not in x_groups:
                    load_group(gi + 1)
            x_g = x_groups[gi]
            sgrp = sgrps[gi]
            xs = x_g[:, :, u * PB:(u + 1) * PB, go * T:(go + 1) * T]

            # ---- PE phase: xr = x + W @ spk_prev + beta*v_init@t0 ----
            # (the leading x-identity matmul was issued one block ahead)
            xr = None
            if n > 0:
                xr = xr_next[u]
                for cjs in ((0, 1), (2, 3)):
                    for v in range(3):
                        for ci in range(CH):
                            for cj in cjs:
                                nc.tensor.matmul(
                                    out=xr[:, ci],
                                    lhsT=wt_t[:, v * 16 + cj * CH + ci],
                                    rhs=spk_prev[u][:, cj],
                                    start=False, stop=False)
                nc.tensor.matmul(
                    out=xr[:, :, :, 0:1], lhsT=id_t[:],
                    rhs=vb_t[u].rearrange("p c u v -> p (c u v)"),
                    start=False, stop=True)
            if n < NB - 1:
                # pre-start next block's PSUM group with the x injection
                ngi, ngo = divmod(n + 1, GRP)
                xg_n = x_groups[ngi]
                xs_n = xg_n[:, :, u * PB:(u + 1) * PB, ngo * T:(ngo + 1) * T]
                xrn = psum.tile([128, CH, PB, T], f32, tag=f"xr{u}")
                nc.tensor.matmul(out=xrn[:], lhsT=id_t[:], rhs=xs_n,
                                 start=True, stop=False)
                xr_next[u] = xrn

            # ---- DVE chain ----
            mem_t = work.tile([128, CH, PB, T], f32, tag=f"mem{u}")
            spk_t = spkp.tile([128, CH, PB, T], bf16, tag=f"spk{u}")

            cur_t = work.tile([128, CH, PB, T], f32, tag=f"cur{u}")
            if n == 0:
                nc.scalar.copy(out=cur_t[:], in_=xs)
            else:
                nc.vector.scalar_tensor_tensor(
                    out=pf(cur_t), in0=pf(q_t[u]), scalar=1.0,
                    in1=pf(xr), op0=Alu.min, op1=Alu.mult)

            nc.vector.tensor_tensor_scan(
                out=pf(mem_t), data0=betaseg_t[:], data1=pf(cur_t),
                initial=0.0, op0=Alu.mult, op1=Alu.add)

            # fs into the 33-wide spacer layout (col 32 stays 0 forever)
            f33u = fs33[u]
            s33u = s33[u]
            nc.vector.scalar_tensor_tensor(
                out=f33u[:, :, 0:T],
                in0=mem_t[:].rearrange("p c u t -> p (c u) t"), scalar=1.0,
                in1=thr_t[u][:].rearrange("p c u t -> p (c u) t"),
                op0=Alu.subtract, op1=Alu.is_gt)

            # or-scan across 33-wide segments; the spacer (d0=0, d1=0)
            # resets state so each segment sees a zero t=-1 column in s33
            nc.vector.tensor_tensor_scan(
                out=s33u[:, 1:1 + SEGF], data0=seg33_t[:],
                data1=f33u[:].rearrange("p s t -> p (s t)"),
                initial=0.0, op0=Alu.mult, op1=Alu.logical_or)

            # spk = first-spike edge in ONE xor: the shifted view supplies
            # each segment's spacer zero as its t=-1 value
            sv = s33u[:, 0:SEGF].rearrange("p (s t) -> p s t", t=T + 1)
            svs = s33u[:, 1:1 + SEGF].rearrange("p (s t) -> p s t", t=T + 1)
            nc.vector.tensor_tensor(
                out=spk_t[:].rearrange("p c u t -> p (c u) t"),
                in0=sv[:, :, 0:T], in1=svs[:, :, 0:T], op=Alu.logical_xor)
            spk_prev[u] = spk_t

            # ---- deferred state for next block ----
            if n < NB - 1:
                # v_init path first: it gates the next PE burst's vinit matmul
                ns_f = small.tile([128, NSEG, 1], f32, tag=f"nsf{u}")
                nc.gpsimd.tensor_single_scalar(
                    out=ns_f[:], in_=svs[:, :, T - 1:T], scalar=0.0,
                    op=Alu.is_equal)
                vinit_new = small.tile([128, CH, PB, 1], f32, tag=f"vi{u}")
                nc.gpsimd.tensor_tensor(
                    out=vinit_new[:].rearrange("p c u v -> p (c u) v"),
                    in0=mem_t[:, :, :, T - 1:T]
                        .rearrange("p c u v -> p (c u) v"),
                    in1=ns_f[:], op=Alu.mult)
                vb_new = small.tile([128, CH, PB, 1], f32, tag=f"vb{u}")
                nc.gpsimd.tensor_tensor(
                    out=vb_new[:], in0=vinit_new[:],
                    in1=betat_t.broadcast_to([128, CH, PB, 1]), op=Alu.mult)
                vb_t[u] = vb_new

                # gate for the next block: q = (s_last == 0) | s.
                # nsr = 1 - s_last materialized dense on the idle Act engine
                # so the q TT gets the 2x packed-bf16 DVE mode
                nsr = spkp.tile([128, NSEG, T], bf16, tag=f"nsr{u}")
                nc.scalar.activation(
                    out=nsr[:],
                    in_=svs[:, :, T - 1:T].broadcast_to([128, NSEG, T]),
                    func=mybir.ActivationFunctionType.Identity,
                    scale=-1.0, bias=1.0)
                qn = spkp.tile([128, CH, PB, T], bf16, tag=f"q{u}")
                nc.vector.tensor_tensor(
                    out=qn[:].rearrange("p c u t -> p (c u) t"), in0=nsr[:],
                    in1=svs[:, :, 0:T], op=Alu.logical_or)

                pdec_t = work.tile([128, CH, PB, T], f32, tag=f"pdec{u}")
                nc.vector.tensor_tensor_scan(
                    out=pf(pdec_t), data0=pseg_t[:], data1=pf(spk_t),
                    initial=0.0, op0=Alu.mult, op1=Alu.add)

                # adaptation state a' = pdec_last/p + a*p^32
                a_new = small.tile([128, CH, PB, 1], f32, tag=f"a{u}")
                u_t = small.tile([128, CH, PB, 1], f32, tag=f"u{u}")
                nc.gpsimd.tensor_tensor(
                    out=u_t[:], in0=pdec_t[:, :, :, T - 1:T],
                    in1=invp_t.broadcast_to([128, CH, PB, 1]), op=Alu.mult)
                if n == 0:
                    nc.gpsimd.tensor_copy(out=a_new[:], in_=u_t[:])
                else:
                    v_t = small.tile([128, CH, PB, 1], f32, tag=f"v{u}")
                    nc.gpsimd.tensor_tensor(
                        out=v_t[:], in0=a_t[u],
                        in1=p32_t.broadcast_to([128, CH, PB, 1]),
                        op=Alu.mult)
                    nc.gpsimd.tensor_tensor(out=a_new[:], in0=u_t[:],
                                            in1=v_t[:], op=Alu.add)

                thr_new = work.tile([128, CH, PB, T], f32, tag=f"thr{u}")
                nc.gpsimd.tensor_tensor(
                    out=thr_new[:], in0=a_new.broadcast_to([128, CH, PB, T]),
                    in1=bp1_t[:], op=Alu.mult)

                q_t[u], thr_t[u], a_t[u] = qn, thr_new, a_new

            # output regroup (bf16 0/1 -> f32), lowest urgency
            nc.scalar.copy(out=sgrp[:, :, u * PB:(u + 1) * PB, go],
                           in_=spk_t[:])

            # pair 1 trails: it retires the group's out-store
            last_grp = (gi + 1) * GRP >= NB
            if u == NP - 1 and last_grp and go == GRP - 2:
                # drain shortening: store the last group's first 7 blocks as
                # soon as block 30 retires, overlapping block 31's compute;
                # only the tiny final block remains on the tail
                for ci in range(CH):
                    nc.scalar.dma_start(
                        out=dram_block_ap(out_d, ci, gi * GRP, (GRP - 1) * T),
                        in_=sgrp[:, ci, :, 0:GRP - 1])
            if u == NP - 1 and go == GRP - 1:
                # out-stores on the Activation HWDGE queue, separate from
                # the SP queue carrying the x group loads
                if last_grp:
                    for ci in range(CH):
                        nc.scalar.dma_start(
                            out=dram_block_ap(out_d, ci, gi * GRP + GRP - 1, T),
                            in_=sgrp[:, ci, :, GRP - 1:GRP])
                else:
                    for ci in range(CH):
                        nc.scalar.dma_start(
                            out=dram_block_ap(out_d, ci, gi * GRP, GRP * T),
                            in_=sgrp[:, ci])
                del sgrps[gi]
                if gi > 0:
                    x_groups.pop(gi - 1, None)

        # software-pipeline the two independent batch pairs one block apart
        # so their chain and matmul phases run in anti-phase
        for k in range(NB + 1):
            if k < NB:
                issue_block(0, k)
            if k >= 1:
                issue_block(1, k - 1)

    with tile.TileContext(nc) as tc:
        kern(tc)
    nc.compile()
    return nc


def _host_tables(beta_raw, rec_weight, p_raw, b_raw):
    f = np.float32
    W = rec_weight.astype(f)
    beta = np.clip(beta_raw.astype(f), f(0.001), f(0.999))
    p = np.clip(np.abs(p_raw.astype(f)), f(0.0), f(0.999))
    bb = np.clip(np.abs(b_raw.astype(f)), f(0.001), f(1.0))
    p_pow = (p[:, None] ** np.arange(1, T + 1, dtype=f)).astype(f)   # (C,T)
    BP1 = (bb[:, None] * p_pow).astype(f)
    p32 = np.ascontiguousarray(p_pow[:, -1])
    invp = (f(1.0) / p).astype(f)

    def per_ct_pair(vals_ct):  # (C,T) -> (128, CH*PB*T), replicated over pair-b
        v = vals_ct.reshape(CH, 128, T)
        out = np.zeros((128, CH, PB, T), f)
        out[:] = v.transpose(1, 0, 2)[:, :, None, :]
        return np.ascontiguousarray(out.reshape(128, PFREE))

    seg33v = np.ones((T + 1,), f)
    seg33v[0] = 0.0
    seg33v[T] = 0.0
    seg33 = np.ascontiguousarray(
        np.broadcast_to(np.tile(seg33v, NSEG)[None, :], (128, SEGF)).astype(f))

    t0mask = np.ones((1, T), f)
    t0mask[0, 0] = 0.0
    betaseg = per_ct_pair((beta[:, None] * t0mask).astype(f))
    pseg = per_ct_pair((p[:, None] * t0mask).astype(f))
    seg01 = per_ct_pair(np.broadcast_to(t0mask, (C, T)).astype(f))
    bp1 = per_ct_pair(BP1)

    def per_c(vals_c):  # (C,) -> (128, CH)
        return np.ascontiguousarray(vals_c.reshape(CH, 128).T)

    # wt[cj_hi*CH + ci_hi][cj_lo, ci_lo] = W[ci_hi*128+ci_lo, cj_hi*128+cj_lo]
    import ml_dtypes
    W4 = W.reshape(CH, 128, CH, 128)
    wt16 = np.ascontiguousarray(
        W4.transpose(2, 0, 3, 1).reshape(16, 128, 128))
    # exact 3-way bf16 decomposition: w1+w2+w3 == W to ~2^-27 relative
    w1 = wt16.astype(ml_dtypes.bfloat16)
    r1 = wt16 - w1.astype(f)
    w2 = r1.astype(ml_dtypes.bfloat16)
    r2 = r1 - w2.astype(f)
    w3 = r2.astype(ml_dtypes.bfloat16)
    wt = np.ascontiguousarray(
        np.concatenate([w1, w2, w3], axis=0).transpose(1, 0, 2))
    ident = np.eye(128, dtype=f)
    return dict(wt=wt, betat=per_c(beta), ident=ident, betaseg=betaseg,
                pseg=pseg, seg33=seg33, bp1=bp1,
                p32t=per_c(p32), invpt=per_c(invp))


def kernel(x, beta_raw, rec_weight, p_raw, b_raw):
    global _compiled
    from concourse.bass_utils import run_bass_kernel_spmd

    if _compiled is None:
        _compiled = _build_program()
    nc = _compiled

    tables = _host_tables(np.asarray(beta_raw), np.asarray(rec_weight),
                          np.asarray(p_raw), np.asarray(b_raw))
    x = np.ascontiguousarray(np.asarray(x).astype(np.float32))
    in_maps = []
    for k in range(NCORES):
        m = {"x_sh": np.ascontiguousarray(x[k * BPC:(k + 1) * BPC])}
        m.update(tables)
        in_maps.append(m)
    res = run_bass_kernel_spmd(nc, in_maps, list(range(NCORES)))
    out = np.concatenate([res.results[k]["out"] for k in range(NCORES)], axis=0)
    return out.astype(np.float32)


# revision 69
# speedup vs baseline: 1.0565x; 1.0565x over previous
"""Trainium2 Bass kernel for the blockwise spiking network (nn_Blocks_86096914416140).

Sharding: data-parallel over batch B=32 across 8 NeuronCores (4 batches/core),
all parameters replicated, zero collectives.

v1 design: the 4 batches per core form 2 independent pairs (the recurrence
couples channels, never batches), software-pipelined one block apart so the
serial spike->matmul->membrane chain of one pair overlaps the other's work:

  PE    : xr_u = x + W@spk_u(n-1) + beta*v_init@t0  (x-identity matmul
          pre-issued one block ahead so the burst is just W + vinit)
  DVE   : cur = q*xr ; mem = scan(beta,cur) ; fs = (mem-1)>thr into a
          33-wide segment layout with zero spacer columns ;
          s = or-scan over the 33-layout [bf16] ;
          spk = s xor shift(s) in ONE 2x-mode bf16 op (the shifted view
          reads each segment's spacer zero as its t=-1 value) ;
          q = nsr|s [bf16 2x] ; pdec = scan(p, spk)
  Pool  : ns/v_init/adaptation smalls ; thr' = a'*b*p^{t+1}
          (GpSimd has no ScalarTensorTensor/scan opcodes on TRN2 - only
          tensor_tensor / tensor_single_scalar / copy forms are legal)
  Act   : nsr = 1-s_last ; spike regroup to f32 out groups ; out stores
          ride the Act HWDGE queue, x loads the SP queue

All float arithmetic is bit-identical to the flips=0 baseline (the dynamics
are chaotic: mem perturbations ~1e-5 cascade to >5000 output flips, so the
fp32 scan chain, the exact 3-way bf16 weight split, and the PSUM accumulation
order are all preserved).  The replaced pieces (count-scan -> or-scan,
spk = first-spike edge via xor, gate = s|ns) are exact boolean identities.
Measured: 167096 ns cost-model exec (baseline 196394), flips=0 on HW.
"""

import numpy as np

B, C, T_LEN, T = 32, 512, 1024, 32
NB = T_LEN // T          # 32 blocks
NCORES = 8
BPC = B // NCORES        # 4 batches per core
CH = C // 128            # 4 channel tiles
NP = 2                   # batch pairs per core
PB = BPC // NP           # batches per pair (2)
PFREE = CH * PB * T      # 256 free elements per pair tile
NSEG = CH * PB           # 8 (c,b) segments per pair tile
SEGF = NSEG * (T + 1)    # 264: segment layout with a zero spacer column
GRP = 4                  # blocks per x-load group

_compiled = None


def _build_program():
    import concourse.bass as bass
    import concourse.bacc as bacc
    import concourse.tile as tile
    from concourse import mybir
    from concourse._compat import with_exitstack
    from contextlib import ExitStack

    f32 = mybir.dt.float32
    bf16 = mybir.dt.bfloat16
    Alu = mybir.AluOpType

    nc = bacc.Bacc()
    x_d = nc.declare_dram_parameter("x_sh", [BPC, C, T_LEN], f32, isOutput=False)
    wt_d = nc.declare_dram_parameter("wt", [128, 48, 128], bf16, isOutput=False)
    betat_d = nc.declare_dram_parameter("betat", [128, CH], f32, isOutput=False)
    ident_d = nc.declare_dram_parameter("ident", [128, 128], f32, isOutput=False)
    betaseg_d = nc.declare_dram_parameter("betaseg", [128, PFREE], f32, isOutput=False)
    pseg_d = nc.declare_dram_parameter("pseg", [128, PFREE], f32, isOutput=False)
    seg33_d = nc.declare_dram_parameter("seg33", [128, SEGF], f32, isOutput=False)
    bp1_d = nc.declare_dram_parameter("bp1", [128, PFREE], f32, isOutput=False)
    p32_d = nc.declare_dram_parameter("p32t", [128, CH], f32, isOutput=False)
    invp_d = nc.declare_dram_parameter("invpt", [128, CH], f32, isOutput=False)
    out_d = nc.declare_dram_parameter("out", [BPC, C, T_LEN], f32, isOutput=True)

    def dram_block_ap(handle, ci, nblk, nt):
        # [128 part = c_lo, (b, t)] view of dram[b, ci*128:(ci+1)*128, nblk*T:...]
        return bass.AP(
            tensor=handle,
            offset=ci * 128 * T_LEN + nblk * T,
            ap=[[T_LEN, 128], [C * T_LEN, BPC], [1, nt]],
        )

    def pf(t4):
        # [128, CH, PB, T] pair tile -> [128, 256] dense view
        return t4.rearrange("p c u t -> p (c u t)")

    @with_exitstack
    def kern(ctx: ExitStack, tc: tile.TileContext):
        consts = ctx.enter_context(tc.tile_pool(name="consts", bufs=1))
        xpool = ctx.enter_context(tc.tile_pool(name="xpool", bufs=3))
        work = ctx.enter_context(tc.tile_pool(name="work", bufs=3))
        spkp = ctx.enter_context(tc.tile_pool(name="spkp", bufs=3))
        small = ctx.enter_context(tc.tile_pool(name="small", bufs=3))
        psum = ctx.enter_context(tc.tile_pool(name="psum", bufs=3, space="PSUM"))

        dma = nc.sync

        x_groups = {}
        sgrps = {}

        def load_group(g):
            xg = xpool.tile([128, CH, BPC, GRP * T], f32, tag="xg")
            for ci in range(CH):
                dma.dma_start(out=xg[:, ci],
                              in_=dram_block_ap(x_d, ci, g * GRP, GRP * T))
            x_groups[g] = xg

        load_group(0)

        # SP-queue order tracks first-use time: block0 chain tables, then
        # the v1 weight chunk (first matmuls), remaining tables, then v2/v3.
        # wt is partition-major in DRAM (contiguous 4KB runs, no small-
        # element DMA penalty) and split in 3 so the first burst can start
        # before the whole 12KB/partition transfer lands.
        betaseg_t = consts.tile([128, PFREE], f32, tag="betaseg")
        dma.dma_start(out=betaseg_t[:], in_=betaseg_d[:])
        seg33_t = consts.tile([128, SEGF], f32, tag="seg33")
        dma.dma_start(out=seg33_t[:], in_=seg33_d[:])
        wt_t = consts.tile([128, 48, 128], bf16, tag="wt")
        dma.dma_start(out=wt_t[:, 0:16], in_=wt_d[:, 0:16])
        id_t = consts.tile([128, 128], f32, tag="ident")
        nc.scalar.dma_start(out=id_t[:], in_=ident_d[:])
        betat_t = consts.tile([128, CH, 1, 1], f32, tag="betat")
        dma.dma_start(out=betat_t[:],
                      in_=betat_d[:].rearrange("p (c u v) -> p c u v", u=1, v=1))
        pseg_t = consts.tile([128, PFREE], f32, tag="pseg")
        dma.dma_start(out=pseg_t[:], in_=pseg_d[:])
        bp1_t = consts.tile([128, CH, PB, T], f32, tag="bp1")
        dma.dma_start(out=bp1_t[:],
                      in_=bp1_d[:].rearrange("p (c u t) -> p c u t", c=CH, u=PB))
        p32_t = consts.tile([128, CH, 1, 1], f32, tag="p32t")
        dma.dma_start(out=p32_t[:],
                      in_=p32_d[:].rearrange("p (c u v) -> p c u v", u=1, v=1))
        invp_t = consts.tile([128, CH, 1, 1], f32, tag="invpt")
        dma.dma_start(out=invp_t[:],
                      in_=invp_d[:].rearrange("p (c u v) -> p c u v", u=1, v=1))
        dma.dma_start(out=wt_t[:, 16:32], in_=wt_d[:, 16:32])
        dma.dma_start(out=wt_t[:, 32:48], in_=wt_d[:, 32:48])

        thr0_t = consts.tile([128, CH, PB, T], f32, tag="thr0")
        nc.vector.memset(thr0_t[:], 0.0)
        fs33 = []
        s33 = []
        for u in range(NP):
            f33u = consts.tile([128, NSEG, T + 1], f32, tag=f"fs33_{u}")
            nc.vector.memset(f33u[:], 0.0)
            fs33.append(f33u)
            s33u = consts.tile([128, 1 + SEGF], bf16, tag=f"s33_{u}")
            nc.vector.memset(s33u[:], 0.0)
            s33.append(s33u)

        # persistent per-pair state (rebound each block)
        spk_prev = [None, None]
        q_t = [None, None]
        thr_t = [thr0_t, thr0_t]
        vb_t = [None, None]
        a_t = [None, None]
        xr_next = [None, None]   # next block's PSUM tile, x pre-injected

        def issue_block(u, n):
            gi, go = divmod(n, GRP)
            if u == 0:
                # pair 0 leads: it owns group prefetch and sgrp allocation
                if go == 0 and gi not in sgrps:
                    sgrp_new = xpool.tile([128, CH, BPC, GRP, T], f32,
                                          tag="sgrp")
                    sgrps[gi] = sgrp_new
                if go == 2 and (gi + 1) * GRP < NB and gi + 1 not in x_groups:
                    load_group(gi + 1)
            x_g = x_groups[gi]
            sgrp = sgrps[gi]
            xs = x_g[:, :, u * PB:(u + 1) * PB, go * T:(go + 1) * T]

            # ---- PE phase: xr = x + W @ spk_prev + beta*v_init@t0 ----
            # (the leading x-identity matmul was issued one block ahead)
            xr = None
            if n > 0:
                xr = xr_next[u]
                for cjs in ((0, 1), (2, 3)):
                    for v in range(3):
                        for ci in range(CH):
                            for cj in cjs:
                                nc.tensor.matmul(
                                    out=xr[:, ci],
                                    lhsT=wt_t[:, v * 16 + cj * CH + ci],
                                    rhs=spk_prev[u][:, cj],
                                    start=False, stop=False)
                nc.tensor.matmul(
                    out=xr[:, :, :, 0:1], lhsT=id_t[:],
                    rhs=vb_t[u].rearrange("p c u v -> p (c u v)"),
                    start=False, stop=True)
            if n < NB - 1:
                # pre-start next block's PSUM group with the x injection
                ngi, ngo = divmod(n + 1, GRP)
                xg_n = x_groups[ngi]
                xs_n = xg_n[:, :, u * PB:(u + 1) * PB, ngo * T:(ngo + 1) * T]
                xrn = psum.tile([128, CH, PB, T], f32, tag=f"xr{u}")
                nc.tensor.matmul(out=xrn[:], lhsT=id_t[:], rhs=xs_n,
                                 start=True, stop=False)
                xr_next[u] = xrn

            # ---- DVE chain ----
            mem_t = work.tile([128, CH, PB, T], f32, tag=f"mem{u}")
            spk_t = spkp.tile([128, CH, PB, T], bf16, tag=f"spk{u}")

            cur_t = work.tile([128, CH, PB, T], f32, tag=f"cur{u}")
            if n == 0:
                nc.scalar.copy(out=cur_t[:], in_=xs)
            else:
                nc.vector.scalar_tensor_tensor(
                    out=pf(cur_t), in0=pf(q_t[u]), scalar=1.0,
                    in1=pf(xr), op0=Alu.min, op1=Alu.mult)

            nc.vector.tensor_tensor_scan(
                out=pf(mem_t), data0=betaseg_t[:], data1=pf(cur_t),
                initial=0.0, op0=Alu.mult, op1=Alu.add)

            # fs into the 33-wide spacer layout (col 32 stays 0 forever)
            f33u = fs33[u]
            s33u = s33[u]
            nc.vector.scalar_tensor_tensor(
                out=f33u[:, :, 0:T],
                in0=mem_t[:].rearrange("p c u t -> p (c u) t"), scalar=1.0,
                in1=thr_t[u][:].rearrange("p c u t -> p (c u) t"),
                op0=Alu.subtract, op1=Alu.is_gt)

            # or-scan across 33-wide segments; the spacer (d0=0, d1=0)
            # resets state so each segment sees a zero t=-1 column in s33
            nc.vector.tensor_tensor_scan(
                out=s33u[:, 1:1 + SEGF], data0=seg33_t[:],
                data1=f33u[:].rearrange("p s t -> p (s t)"),
                initial=0.0, op0=Alu.mult, op1=Alu.logical_or)

            # spk = first-spike edge in ONE xor: the shifted view supplies
            # each segment's spacer zero as its t=-1 value
            sv = s33u[:, 0:SEGF].rearrange("p (s t) -> p s t", t=T + 1)
            svs = s33u[:, 1:1 + SEGF].rearrange("p (s t) -> p s t", t=T + 1)
            nc.vector.tensor_tensor(
                out=spk_t[:].rearrange("p c u t -> p (c u) t"),
                in0=sv[:, :, 0:T], in1=svs[:, :, 0:T], op=Alu.logical_xor)
            spk_prev[u] = spk_t

            # ---- deferred state for next block ----
            if n < NB - 1:
                # v_init path first: it gates the next PE burst's vinit matmul
                ns_f = small.tile([128, NSEG, 1], f32, tag=f"nsf{u}")
                nc.gpsimd.tensor_single_scalar(
                    out=ns_f[:], in_=svs[:, :, T - 1:T], scalar=0.0,
                    op=Alu.is_equal)
                vinit_new = small.tile([128, CH, PB, 1], f32, tag=f"vi{u}")
                nc.gpsimd.tensor_tensor(
                    out=vinit_new[:].rearrange("p c u v -> p (c u) v"),
                    in0=mem_t[:, :, :, T - 1:T]
                        .rearrange("p c u v -> p (c u) v"),
                    in1=ns_f[:], op=Alu.mult)
                vb_new = small.tile([128, CH, PB, 1], f32, tag=f"vb{u}")
                nc.gpsimd.tensor_tensor(
                    out=vb_new[:], in0=vinit_new[:],
                    in1=betat_t.broadcast_to([128, CH, PB, 1]), op=Alu.mult)
                vb_t[u] = vb_new

                # gate for the next block: q = (s_last == 0) | s.
                # nsr = 1 - s_last materialized dense on the idle Act engine
                # so the q TT gets the 2x packed-bf16 DVE mode
                nsr = spkp.tile([128, NSEG, T], bf16, tag=f"nsr{u}")
                nc.scalar.activation(
                    out=nsr[:],
                    in_=svs[:, :, T - 1:T].broadcast_to([128, NSEG, T]),
                    func=mybir.ActivationFunctionType.Identity,
                    scale=-1.0, bias=1.0)
                qn = spkp.tile([128, CH, PB, T], bf16, tag=f"q{u}")
                nc.vector.tensor_tensor(
                    out=qn[:].rearrange("p c u t -> p (c u) t"), in0=nsr[:],
                    in1=svs[:, :, 0:T], op=Alu.logical_or)

                pdec_t = work.tile([128, CH, PB, T], f32, tag=f"pdec{u}")
                nc.vector.tensor_tensor_scan(
                    out=pf(pdec_t), data0=pseg_t[:], data1=pf(spk_t),
                    initial=0.0, op0=Alu.mult, op1=Alu.add)

                # adaptation state a' = pdec_last/p + a*p^32
                a_new = small.tile([128, CH, PB, 1], f32, tag=f"a{u}")
                u_t = small.tile([128, CH, PB, 1], f32, tag=f"u{u}")
                nc.gpsimd.tensor_tensor(
                    out=u_t[:], in0=pdec_t[:, :, :, T - 1:T],
                    in1=invp_t.broadcast_to([128, CH, PB, 1]), op=Alu.mult)
                if n == 0:
                    nc.gpsimd.tensor_copy(out=a_new[:], in_=u_t[:])
                else:
                    v_t = small.tile([128, CH, PB, 1], f32, tag=f"v{u}")
                    nc.gpsimd.tensor_tensor(
                        out=v_t[:], in0=a_t[u],
                        in1=p32_t.broadcast_to([128, CH, PB, 1]),
                        op=Alu.mult)
                    nc.gpsimd.tensor_tensor(out=a_new[:], in0=u_t[:],
                                            in1=v_t[:], op=Alu.add)

                thr_new = work.tile([128, CH, PB, T], f32, tag=f"thr{u}")
                nc.gpsimd.tensor_tensor(
                    out=thr_new[:], in0=a_new.broadcast_to([128, CH, PB, T]),
                    in1=bp1_t[:], op=Alu.mult)

                q_t[u], thr_t[u], a_t[u] = qn, thr_new, a_new

            # output regroup (bf16 0/1 -> f32), lowest urgency
            nc.scalar.copy(out=sgrp[:, :, u * PB:(u + 1) * PB, go],
                           in_=spk_t[:])

            # pair 1 trails: it retires the group's out-store
            last_grp = (gi + 1) * GRP >= NB
            if u == NP - 1 and last_grp and go == GRP - 2:
                # drain shortening: store the last group's first 7 blocks as
                # soon as block 30 retires, overlapping block 31's compute;
                # only the tiny final block remains on the tail
                for ci in range(CH):
                    nc.scalar.dma_start(
                        out=dram_block_ap(out_d, ci, gi * GRP, (GRP - 1) * T),
                        in_=sgrp[:, ci, :, 0:GRP - 1])
            if u == NP - 1 and go == GRP - 1:
                # out-stores on the Activation HWDGE queue, separate from
                # the SP queue carrying the x group loads
                if last_grp:
                    for ci in range(CH):
                        nc.scalar.dma_start(
                            out=dram_block_ap(out_d, ci, gi * GRP + GRP - 1, T),
                            in_=sgrp[:, ci, :, GRP - 1:GRP])
                else:
                    for ci in range(CH):
                        nc.scalar.dma_start(
                            out=dram_block_ap(out_d, ci, gi * GRP, GRP * T),
                            in_=sgrp[:, ci])
                del sgrps[gi]
                if gi > 0:
                    x_groups.pop(gi - 1, None)

        # software-pipeline the two independent batch pairs one block apart
        # so their chain and matmul phases run in anti-phase
        for k in range(NB + 1):
            if k < NB:
                issue_block(0, k)
            if k >= 1:
                issue_block(1, k - 1)

    with tile.TileContext(nc) as tc:
        kern(tc)
    nc.compile()
    return nc


def _host_tables(beta_raw, rec_weight, p_raw, b_raw):
    f = np.float32
    W = rec_weight.astype(f)
    beta = np.clip(beta_raw.astype(f), f(0.001), f(0.999))
    p = np.clip(np.abs(p_raw.astype(f)), f(0.0), f(0.999))
    bb = np.clip(np.abs(b_raw.astype(f)), f(0.001), f(1.0))
    p_pow = (p[:, None] ** np.arange(1, T + 1, dtype=f)).astype(f)   # (C,T)
    BP1 = (bb[:, None] * p_pow).astype(f)
    p32 = np.ascontiguousarray(p_pow[:, -1])
    invp = (f(1.0) / p).astype(f)

    def per_ct_pair(vals_ct):  # (C,T) -> (128, CH*PB*T), replicated over pair-b
        v = vals_ct.reshape(CH, 128, T)
        out = np.zeros((128, CH, PB, T), f)
        out[:] = v.transpose(1, 0, 2)[:, :, None, :]
        return np.ascontiguousarray(out.reshape(128, PFREE))

    seg33v = np.ones((T + 1,), f)
    seg33v[0] = 0.0
    seg33v[T] = 0.0
    seg33 = np.ascontiguousarray(
        np.broadcast_to(np.tile(seg33v, NSEG)[None, :], (128, SEGF)).astype(f))

    t0mask = np.ones((1, T), f)
    t0mask[0, 0] = 0.0
    betaseg = per_ct_pair((beta[:, None] * t0mask).astype(f))
    pseg = per_ct_pair((p[:, None] * t0mask).astype(f))
    seg01 = per_ct_pair(np.broadcast_to(t0mask, (C, T)).astype(f))
    bp1 = per_ct_pair(BP1)

    def per_c(vals_c):  # (C,) -> (128, CH)
        return np.ascontiguousarray(vals_c.reshape(CH, 128).T)

    # wt[cj_hi*CH + ci_hi][cj_lo, ci_lo] = W[ci_hi*128+ci_lo, cj_hi*128+cj_lo]
    import ml_dtypes
    W4 = W.reshape(CH, 128, CH, 128)
    wt16 = np.ascontiguousarray(
        W4.transpose(2, 0, 3, 1).reshape(16, 128, 128))
    # exact 3-way bf16 decomposition: w1+w2+w3 == W to ~2^-27 relative
    w1 = wt16.astype(ml_dtypes.bfloat16)
    r1 = wt16 - w1.astype(f)
    w2 = r1.astype(ml_dtypes.bfloat16)
    r2 = r1 - w2.astype(f)
    w3 = r2.astype(ml_dtypes.bfloat16)
    wt = np.ascontiguousarray(
        np.concatenate([w1, w2, w3], axis=0).transpose(1, 0, 2))
    ident = np.eye(128, dtype=f)
    return dict(wt=wt, betat=per_c(beta), ident=ident, betaseg=betaseg,
                pseg=pseg, seg33=seg33, bp1=bp1,
                p32t=per_c(p32), invpt=per_c(invp))


def kernel(x, beta_raw, rec_weight, p_raw, b_raw):
    global _compiled
    from concourse.bass_utils import run_bass_kernel_spmd

    if _compiled is None:
        _compiled = _build_program()
    nc = _compiled

    tables = _host_tables(np.asarray(beta_raw), np.asarray(rec_weight),
                          np.asarray(p_raw), np.asarray(b_raw))
    x = np.ascontiguousarray(np.asarray(x).astype(np.float32))
    in_maps = []
    for k in range(NCORES):
        m = {"x_sh": np.ascontiguousarray(x[k * BPC:(k + 1) * BPC])}
        m.update(tables)
        in_maps.append(m)
    res = run_bass_kernel_spmd(nc, in_maps, list(range(NCORES)))
    out = np.concatenate([res.results[k]["out"] for k in range(NCORES)], axis=0)
    return out.astype(np.float32)
